# revision 12
# baseline (speedup 1.0000x reference)
"""LoRALinear kernel for Trainium2 (8 NeuronCores, SPMD data-parallel).

Computes out = x @ W.T + b + SCALE*((x@gA.T)@gB.T + (x@lA.T)@lB.T)
  x: [8, 2048, 1024] f32, W: [4096, 1024], b: [4096]
  gA/lA: [8, 1024], gB/lB: [4096, 8]  ->  out: [8, 2048, 4096] f32

Strategy (per core, one batch of x each). Host ships layout-marshaled
fp16 operands (x.T per core, W.T replicated, zero-padded rank-16 LoRA
factors, bias row replicated to 128 partitions); every FLOP runs on
device:
  1. Warmup: a few full-K dummy matmuls un-throttle the PE HAM clock
     gate (1.2 -> 2.4 GHz) while the first W.T/x.T chunks DMA in
     (split across both HWDGE queues: critical prefix on sync,
     bulk on the scalar engine's queue).
  2. W_effT = W.T + (SCALE*A_cat).T @ B_catT built in place over the
     W.T tiles. The o-quarter needed first is merged entirely on
     PE+ScalarE (LoRA matmul, then identity matmul accumulating W.T
     into the same psum, ScalarE f32->f16 eviction) so the slow DVE
     psum-add never gates the start; later quarters use the DVE add
     path, hidden under main compute.
  3. Main matmul in four o-quarter passes: per 128-row s-tile,
     accumulate psum over 8 k-chunks with the xT chunk stationary
     across 2 o-tiles (LDWEIGHTS amortized). Every W/x chunk is its
     own SBUF tile so dependency tracking stays exact. LoRA prep for
     quarter q+1 is slotted between s-tiles of quarter q. PSUM pools
     are split (prep 2 banks / main 6) to avoid false coupling. DVE
     evicts with fused bias add to fp16; host upcasts to f32.

fp16 operands and fp16 output staging give ~6e-4 absmax relative
error vs the f32 reference; accumulation stays f32 in PSUM.
"""
import numpy as np
from contextlib import ExitStack

import concourse.bass as bass
import concourse.tile as tile
from concourse import bacc, mybir
from concourse.bass import ts, ds
from concourse.bass_utils import run_bass_kernel_spmd
from concourse.masks import make_identity

F32 = mybir.dt.float32
F16 = mybir.dt.float16

N_CORES = 8
B, S, DIN, DOUT, R = 8, 2048, 1024, 4096, 8
SCALE = 16.0 / 8
R2 = 2 * R

P = 128            # partition tile
OTILE = 512        # matmul moving free dim (one PSUM bank of f32)
KT = DIN // P      # 8 k-tiles
OT = DOUT // OTILE # 8 o-tiles
ST = S // P        # 16 s-tiles
SLAB = 512         # xT column slab (4 s-tiles per slab tile)
NSLAB = S // SLAB  # 4
NQ = 4             # o-quarters
NJ = OT // NQ      # 2 o-tiles per quarter
QW = NJ * OTILE    # 1024 columns per quarter


def build_nc():
    nc = bacc.Bacc("TRN2", target_bir_lowering=False, debug=False,
                   num_devices=N_CORES)
    xT = nc.dram_tensor("xT", [DIN, S], F16, kind="ExternalInput").ap()
    WT = nc.dram_tensor("WT", [DIN, DOUT], F16, kind="ExternalInput").ap()
    b_rep = nc.dram_tensor("b_rep", [P, DOUT], F16, kind="ExternalInput").ap()
    A_cat = nc.dram_tensor("A_cat", [P, DIN], F16, kind="ExternalInput").ap()
    B_catT = nc.dram_tensor("B_catT", [P, DOUT], F16, kind="ExternalInput").ap()
    out = nc.dram_tensor("out", [S, DOUT], F16, kind="ExternalOutput").ap()

    with tile.TileContext(nc) as tc:
        with ExitStack() as ctx:
            const = ctx.enter_context(tc.tile_pool(name="const", bufs=1))
            wet_pool = ctx.enter_context(tc.tile_pool(name="wet", bufs=1))
            xt_pool = ctx.enter_context(tc.tile_pool(name="xt", bufs=1))
            out_pool = ctx.enter_context(tc.tile_pool(name="outp", bufs=8))
            psm = ctx.enter_context(tc.tile_pool(name="psm", bufs=6, space="PSUM"))
            psl = ctx.enter_context(tc.tile_pool(name="psl", bufs=2, space="PSUM"))

            # ---- small constants (critical sync queue) ----
            acat = const.tile([P, DIN], F16)
            nc.sync.dma_start(acat[:], A_cat)
            bcatt = const.tile([P, DOUT], F16)
            nc.sync.dma_start(bcatt[:], B_catT)

            # ---- bulk inputs: one SBUF tile per DMA chunk so dependency
            # tracking stays exact (no false whole-tile WAR hazards) ----
            wetq = [[wet_pool.tile([P, QW], F16, tag=f"wetq{k}_{q}",
                                   name=f"wetq{k}_{q}") for q in range(NQ)]
                    for k in range(KT)]
            xts = [[xt_pool.tile([P, SLAB], F16, tag=f"xts{k}_{s}",
                                 name=f"xts{k}_{s}") for s in range(NSLAB)]
                   for k in range(KT)]

            def dma_wet_quarter(q, eng):
                for kt in range(KT):
                    eng.dma_start(wetq[kt][q][:],
                                  WT[ds(kt * P, P), ds(q * QW, QW)])

            def dma_x_slab(sl, eng):
                for kt in range(KT):
                    eng.dma_start(xts[kt][sl][:],
                                  xT[ds(kt * P, P), ts(sl, SLAB)])

            # critical prefix on the sync HWDGE queue
            dma_wet_quarter(0, nc.sync)
            dma_x_slab(0, nc.sync)
            bias16 = const.tile([P, DOUT], F16)
            nc.sync.dma_start(bias16[:], b_rep)
            dma_x_slab(1, nc.sync)
            # bulk on the scalar engine's HWDGE queue, in parallel
            dma_wet_quarter(1, nc.scalar)
            dma_wet_quarter(2, nc.scalar)
            dma_x_slab(2, nc.scalar)
            dma_x_slab(3, nc.scalar)
            dma_wet_quarter(3, nc.scalar)

            # ---- HAM warmup: full-K dummy matmuls un-throttle the PE
            # clock gate while the first real chunks land ----
            warm = const.tile([P, OTILE], F16)
            nc.vector.memset(warm[:], 1.0)
            ident_h = const.tile([P, P], F16)
            make_identity(nc, ident_h)
            psw = psm.tile([P, OTILE], F32, tag="psm")
            NWARM = 12
            for i in range(NWARM):
                nc.tensor.matmul(psw[:], warm[:, ds(0, P)], warm[:],
                                 start=(i == 0), stop=(i == NWARM - 1))
            warm_sink = const.tile([P, 1], F32)
            nc.vector.tensor_copy(warm_sink[:], psw[:, ds(0, 1)])

            def lora_group_fast(q, kt):
                # W_effT chunk fully on PE+ScalarE: psum = LoRA + W.T via
                # an identity matmul, ScalarE evicts f32->f16 in place.
                # Used for the startup-critical quarter (no DVE pacing).
                for j in range(NJ):
                    ot = q * NJ + j
                    wchunk = wetq[kt][q][:, ts(j, OTILE)]
                    pl = psl.tile([P, OTILE], F32, tag="psl",
                                  name=f"plf{q}_{kt}_{j}")
                    nc.tensor.matmul(pl[:], acat[:, ts(kt, P)],
                                     bcatt[:, ts(ot, OTILE)],
                                     start=True, stop=False)
                    nc.tensor.matmul(pl[:], ident_h[:], wchunk,
                                     start=False, stop=True)
                    nc.scalar.copy(wchunk, pl[:])

            def lora_group(q, kt):
                # DVE-add variant, hidden under main compute
                for j in range(NJ):
                    ot = q * NJ + j
                    pl = psl.tile([P, OTILE], F32, tag="psl",
                                  name=f"pl{q}_{kt}_{j}")
                    nc.tensor.matmul(pl[:], acat[:, ts(kt, P)],
                                     bcatt[:, ts(ot, OTILE)],
                                     start=True, stop=True)
                    wchunk = wetq[kt][q][:, ts(j, OTILE)]
                    nc.vector.tensor_tensor(wchunk, pl[:], wchunk,
                                            mybir.AluOpType.add)

            def main_quarter(q, interleave_lora_q=None):
                # out[s, o-quarter] = x @ W_effT + bias; slot the next
                # quarter's LoRA prep between s-tiles to keep PE dense
                for st in range(ST):
                    if interleave_lora_q is not None and 4 <= st < 4 + KT:
                        lora_group(interleave_lora_q, st - 4)
                    pos = [psm.tile([P, OTILE], F32, tag="psm",
                                    name=f"pos{q}_{st}_{j}")
                           for j in range(NJ)]
                    for kt in range(KT):
                        xchunk = xts[kt][st // 4][:, ds((st % 4) * P, P)]
                        for j in range(NJ):
                            nc.tensor.matmul(pos[j][:], xchunk,
                                             wetq[kt][q][:, ts(j, OTILE)],
                                             start=(kt == 0),
                                             stop=(kt == KT - 1))
                    for j in range(NJ):
                        ot = q * NJ + j
                        osb = out_pool.tile([P, OTILE], F16)
                        nc.vector.tensor_tensor(osb[:], pos[j][:],
                                                bias16[:, ts(ot, OTILE)],
                                                mybir.AluOpType.add)
                        nc.sync.dma_start(out[ts(st, P), ts(ot, OTILE)],
                                          osb[:])

            for kt in range(KT):
                lora_group_fast(0, kt)
            main_quarter(0, interleave_lora_q=1)
            main_quarter(1, interleave_lora_q=2)
            main_quarter(2, interleave_lora_q=3)
            main_quarter(3)

    nc.compile()
    return nc


_NC_CACHE = None


def _get_nc():
    global _NC_CACHE
    if _NC_CACHE is None:
        _NC_CACHE = build_nc()
    return _NC_CACHE


def make_in_maps(x, W, b, global_A, global_B, local_A, local_B):
    x = np.asarray(x, dtype=np.float32)
    W = np.asarray(W, dtype=np.float32)
    b = np.asarray(b, dtype=np.float32)
    xT = np.ascontiguousarray(x.transpose(0, 2, 1).astype(np.float16))
    WT = np.ascontiguousarray(W.T.astype(np.float16))
    # rank-16 LoRA factors, zero-padded to K=128 so the device matmuls
    # run full-array (keeps the PE HAM clock gate warm; padded rows are
    # zero in both operands so the result is exact)
    A_cat = np.zeros((P, DIN), dtype=np.float16)
    A_cat[:R2] = (SCALE * np.concatenate(
        [np.asarray(global_A), np.asarray(local_A)], axis=0)).astype(np.float16)
    B_catT = np.zeros((P, DOUT), dtype=np.float16)
    B_catT[:R2] = np.concatenate(
        [np.asarray(global_B).T, np.asarray(local_B).T], axis=0).astype(np.float16)
    b_rep = np.ascontiguousarray(
        np.broadcast_to(b.astype(np.float16), (P, DOUT)))
    return [
        {"xT": xT[i], "WT": WT, "b_rep": b_rep, "A_cat": A_cat,
         "B_catT": B_catT}
        for i in range(N_CORES)
    ]


def kernel(x, W, b, global_A, global_B, local_A, local_B):
    nc = _get_nc()
    in_maps = make_in_maps(x, W, b, global_A, global_B, local_A, local_B)
    res = run_bass_kernel_spmd(nc, in_maps, list(range(N_CORES))).results
    return np.stack([res[i]["out"].astype(np.float32) for i in range(N_CORES)],
                    axis=0)


# revision 14
# speedup vs baseline: 1.1538x; 1.1538x over previous
"""LoRALinear kernel for Trainium2 (8 NeuronCores, SPMD data-parallel).

Computes out = x @ W.T + b + SCALE*((x@gA.T)@gB.T + (x@lA.T)@lB.T)
  x: [8, 2048, 1024] f32, W: [4096, 1024], b: [4096]
  gA/lA: [8, 1024], gB/lB: [4096, 8]  ->  out: [8, 2048, 4096] f32

Strategy (per core, one batch of x each). Host ships layout-marshaled
fp16 operands (x.T per core, W.T replicated, zero-padded rank-16 LoRA
factors, bias row replicated to 128 partitions); every FLOP runs on
device:
  1. Warmup: a few full-K dummy matmuls un-throttle the PE HAM clock
     gate (1.2 -> 2.4 GHz) while the first W.T/x.T chunks DMA in
     (split across both HWDGE queues: critical prefix on sync,
     bulk on the scalar engine's queue).
  2. W_effT = W.T + (SCALE*A_cat).T @ B_catT built in place over the
     W.T tiles. The o-quarter needed first is merged entirely on
     PE+ScalarE (LoRA matmul, then identity matmul accumulating W.T
     into the same psum, ScalarE f32->f16 eviction) so the slow DVE
     psum-add never gates the start; later quarters use the DVE add
     path, hidden under main compute.
  3. Main matmul in four o-quarter passes: per 128-row s-tile,
     accumulate psum over 8 k-chunks with the xT chunk stationary
     across 2 o-tiles (LDWEIGHTS amortized). Every W/x chunk is its
     own SBUF tile so dependency tracking stays exact. LoRA prep for
     quarter q+1 is slotted between s-tiles of quarter q. PSUM pools
     are split (prep 2 banks / main 6) to avoid false coupling. DVE
     evicts with fused bias add to fp16; host upcasts to f32.

fp16 operands and fp16 output staging give ~6e-4 absmax relative
error vs the f32 reference; accumulation stays f32 in PSUM.
"""
import numpy as np
from contextlib import ExitStack

import concourse.bass as bass
import concourse.tile as tile
from concourse import bacc, mybir
from concourse.bass import ts, ds
from concourse.bass_utils import run_bass_kernel_spmd
from concourse.masks import make_identity

F32 = mybir.dt.float32
F16 = mybir.dt.float16

N_CORES = 8
B, S, DIN, DOUT, R = 8, 2048, 1024, 4096, 8
SCALE = 16.0 / 8
R2 = 2 * R

P = 128            # partition tile
OTILE = 512        # matmul moving free dim (one PSUM bank of f32)
KT = DIN // P      # 8 k-tiles
OT = DOUT // OTILE # 8 o-tiles
ST = S // P        # 16 s-tiles
SLAB = 512         # xT column slab (4 s-tiles per slab tile)
NSLAB = S // SLAB  # 4
NQ = 4             # o-quarters
NJ = OT // NQ      # 2 o-tiles per quarter
QW = NJ * OTILE    # 1024 columns per quarter


def build_nc():
    nc = bacc.Bacc("TRN2", target_bir_lowering=False, debug=False,
                   num_devices=N_CORES)
    xT = nc.dram_tensor("xT", [DIN, S], F16, kind="ExternalInput").ap()
    WT = nc.dram_tensor("WT", [DIN, DOUT], F16, kind="ExternalInput").ap()
    b_rep = nc.dram_tensor("b_rep", [P, DOUT], F16, kind="ExternalInput").ap()
    A_cat = nc.dram_tensor("A_cat", [P, DIN], F16, kind="ExternalInput").ap()
    B_catT = nc.dram_tensor("B_catT", [P, DOUT], F16, kind="ExternalInput").ap()
    out = nc.dram_tensor("out", [S, DOUT], F16, kind="ExternalOutput").ap()

    with tile.TileContext(nc) as tc:
        with ExitStack() as ctx:
            const = ctx.enter_context(tc.tile_pool(name="const", bufs=1))
            wet_pool = ctx.enter_context(tc.tile_pool(name="wet", bufs=1))
            xt_pool = ctx.enter_context(tc.tile_pool(name="xt", bufs=1))
            out_pool = ctx.enter_context(tc.tile_pool(name="outp", bufs=16))
            psm = ctx.enter_context(tc.tile_pool(name="psm", bufs=6, space="PSUM"))
            psl = ctx.enter_context(tc.tile_pool(name="psl", bufs=2, space="PSUM"))

            # ---- small constants (critical sync queue) ----
            acat = const.tile([P, DIN], F16)
            nc.sync.dma_start(acat[:], A_cat)
            bcatt = const.tile([P, DOUT], F16)
            nc.sync.dma_start(bcatt[:], B_catT)

            # ---- bulk inputs: one SBUF tile per DMA chunk so dependency
            # tracking stays exact (no false whole-tile WAR hazards) ----
            wetq = [[wet_pool.tile([P, QW], F16, tag=f"wetq{k}_{q}",
                                   name=f"wetq{k}_{q}") for q in range(NQ)]
                    for k in range(KT)]
            xts = [[xt_pool.tile([P, SLAB], F16, tag=f"xts{k}_{s}",
                                 name=f"xts{k}_{s}") for s in range(NSLAB)]
                   for k in range(KT)]

            def dma_wet_quarter(q, eng):
                for kt in range(KT):
                    eng.dma_start(wetq[kt][q][:],
                                  WT[ds(kt * P, P), ds(q * QW, QW)])

            def dma_x_slab(sl, eng):
                for kt in range(KT):
                    eng.dma_start(xts[kt][sl][:],
                                  xT[ds(kt * P, P), ts(sl, SLAB)])

            # single sync HWDGE queue, critical prefix first (FIFO gives
            # the prefix full bandwidth; issuing bulk DMAs from ScalarE
            # would block ScalarE's FIFO ahead of the prep evictions)
            dma_wet_quarter(0, nc.sync)
            dma_x_slab(0, nc.sync)
            bias16 = const.tile([P, DOUT], F16)
            nc.sync.dma_start(bias16[:], b_rep)
            dma_x_slab(1, nc.sync)
            dma_wet_quarter(1, nc.sync)
            dma_x_slab(2, nc.sync)
            dma_x_slab(3, nc.sync)
            dma_wet_quarter(2, nc.sync)
            dma_wet_quarter(3, nc.sync)

            # ---- HAM warmup: full-K dummy matmuls un-throttle the PE
            # clock gate while the first real chunks land ----
            warm = const.tile([P, OTILE], F16)
            nc.vector.memset(warm[:], 1.0)
            ident_h = const.tile([P, P], F16)
            make_identity(nc, ident_h)
            psw = psm.tile([P, OTILE], F32, tag="psm")
            NWARM = 12
            for i in range(NWARM):
                nc.tensor.matmul(psw[:], warm[:, ds(0, P)], warm[:],
                                 start=(i == 0), stop=(i == NWARM - 1))
            warm_sink = const.tile([P, 1], F32)
            nc.vector.tensor_copy(warm_sink[:], psw[:, ds(0, 1)])

            def lora_group_fast(q, kt):
                # W_effT chunk fully on PE+ScalarE: psum = LoRA + W.T via
                # an identity matmul, ScalarE evicts f32->f16 in place.
                # Used for the startup-critical quarter (no DVE pacing).
                for j in range(NJ):
                    ot = q * NJ + j
                    wchunk = wetq[kt][q][:, ts(j, OTILE)]
                    pl = psl.tile([P, OTILE], F32, tag="psl",
                                  name=f"plf{q}_{kt}_{j}")
                    nc.tensor.matmul(pl[:], acat[:, ts(kt, P)],
                                     bcatt[:, ts(ot, OTILE)],
                                     start=True, stop=False)
                    nc.tensor.matmul(pl[:], ident_h[:], wchunk,
                                     start=False, stop=True)
                    nc.scalar.copy(wchunk, pl[:])

            def lora_group(q, kt):
                # DVE-add variant, hidden under main compute
                for j in range(NJ):
                    ot = q * NJ + j
                    pl = psl.tile([P, OTILE], F32, tag="psl",
                                  name=f"pl{q}_{kt}_{j}")
                    nc.tensor.matmul(pl[:], acat[:, ts(kt, P)],
                                     bcatt[:, ts(ot, OTILE)],
                                     start=True, stop=True)
                    wchunk = wetq[kt][q][:, ts(j, OTILE)]
                    nc.vector.tensor_tensor(wchunk, pl[:], wchunk,
                                            mybir.AluOpType.add)

            def main_quarter(q, interleave_lora_q=None):
                # out[s, o-quarter] = x @ W_effT + bias; slot the next
                # quarter's LoRA prep between s-tiles to keep PE dense
                for st in range(ST):
                    if interleave_lora_q is not None and 4 <= st < 4 + KT:
                        lora_group(interleave_lora_q, st - 4)
                    pos = [psm.tile([P, OTILE], F32, tag="psm",
                                    name=f"pos{q}_{st}_{j}")
                           for j in range(NJ)]
                    for kt in range(KT):
                        xchunk = xts[kt][st // 4][:, ds((st % 4) * P, P)]
                        for j in range(NJ):
                            nc.tensor.matmul(pos[j][:], xchunk,
                                             wetq[kt][q][:, ts(j, OTILE)],
                                             start=(kt == 0),
                                             stop=(kt == KT - 1))
                    for j in range(NJ):
                        ot = q * NJ + j
                        osb = out_pool.tile([P, OTILE], F16)
                        nc.vector.tensor_tensor(osb[:], pos[j][:],
                                                bias16[:, ts(ot, OTILE)],
                                                mybir.AluOpType.add)
                        nc.sync.dma_start(out[ts(st, P), ts(ot, OTILE)],
                                          osb[:])

            for kt in range(KT):
                lora_group_fast(0, kt)
            main_quarter(0, interleave_lora_q=1)
            main_quarter(1, interleave_lora_q=2)
            main_quarter(2, interleave_lora_q=3)
            main_quarter(3)

    nc.compile()
    return nc


_NC_CACHE = None


def _get_nc():
    global _NC_CACHE
    if _NC_CACHE is None:
        _NC_CACHE = build_nc()
    return _NC_CACHE


def make_in_maps(x, W, b, global_A, global_B, local_A, local_B):
    x = np.asarray(x, dtype=np.float32)
    W = np.asarray(W, dtype=np.float32)
    b = np.asarray(b, dtype=np.float32)
    xT = np.ascontiguousarray(x.transpose(0, 2, 1).astype(np.float16))
    WT = np.ascontiguousarray(W.T.astype(np.float16))
    # rank-16 LoRA factors, zero-padded to K=128 so the device matmuls
    # run full-array (keeps the PE HAM clock gate warm; padded rows are
    # zero in both operands so the result is exact)
    A_cat = np.zeros((P, DIN), dtype=np.float16)
    A_cat[:R2] = (SCALE * np.concatenate(
        [np.asarray(global_A), np.asarray(local_A)], axis=0)).astype(np.float16)
    B_catT = np.zeros((P, DOUT), dtype=np.float16)
    B_catT[:R2] = np.concatenate(
        [np.asarray(global_B).T, np.asarray(local_B).T], axis=0).astype(np.float16)
    b_rep = np.ascontiguousarray(
        np.broadcast_to(b.astype(np.float16), (P, DOUT)))
    return [
        {"xT": xT[i], "WT": WT, "b_rep": b_rep, "A_cat": A_cat,
         "B_catT": B_catT}
        for i in range(N_CORES)
    ]


def kernel(x, W, b, global_A, global_B, local_A, local_B):
    nc = _get_nc()
    in_maps = make_in_maps(x, W, b, global_A, global_B, local_A, local_B)
    res = run_bass_kernel_spmd(nc, in_maps, list(range(N_CORES))).results
    return np.stack([res[i]["out"].astype(np.float32) for i in range(N_CORES)],
                    axis=0)


# revision 15
# speedup vs baseline: 1.1619x; 1.0070x over previous
"""LoRALinear kernel for Trainium2 (8 NeuronCores, SPMD data-parallel).

Computes out = x @ W.T + b + SCALE*((x@gA.T)@gB.T + (x@lA.T)@lB.T)
  x: [8, 2048, 1024] f32, W: [4096, 1024], b: [4096]
  gA/lA: [8, 1024], gB/lB: [4096, 8]  ->  out: [8, 2048, 4096] f32

Strategy (per core, one batch of x each). Host ships layout-marshaled
fp16 operands (x.T per core, W.T replicated, zero-padded rank-16 LoRA
factors, bias row replicated to 128 partitions); every FLOP runs on
device:
  1. Warmup: a few full-K dummy matmuls un-throttle the PE HAM clock
     gate (1.2 -> 2.4 GHz) while the first W.T/x.T chunks DMA in
     (split across both HWDGE queues: critical prefix on sync,
     bulk on the scalar engine's queue).
  2. W_effT = W.T + (SCALE*A_cat).T @ B_catT built in place over the
     W.T tiles. The o-quarter needed first is merged entirely on
     PE+ScalarE (LoRA matmul, then identity matmul accumulating W.T
     into the same psum, ScalarE f32->f16 eviction) so the slow DVE
     psum-add never gates the start; later quarters use the DVE add
     path, hidden under main compute.
  3. Main matmul in four o-quarter passes: per 128-row s-tile,
     accumulate psum over 8 k-chunks with the xT chunk stationary
     across 2 o-tiles (LDWEIGHTS amortized). Every W/x chunk is its
     own SBUF tile so dependency tracking stays exact. LoRA prep for
     quarter q+1 is slotted between s-tiles of quarter q. PSUM pools
     are split (prep 2 banks / main 6) to avoid false coupling. DVE
     evicts with fused bias add to fp16; host upcasts to f32.

fp16 operands and fp16 output staging give ~6e-4 absmax relative
error vs the f32 reference; accumulation stays f32 in PSUM.
"""
import numpy as np
from contextlib import ExitStack

import concourse.bass as bass
import concourse.tile as tile
from concourse import bacc, mybir
from concourse.bass import ts, ds
from concourse.bass_utils import run_bass_kernel_spmd
from concourse.masks import make_identity

F32 = mybir.dt.float32
F16 = mybir.dt.float16

N_CORES = 8
B, S, DIN, DOUT, R = 8, 2048, 1024, 4096, 8
SCALE = 16.0 / 8
R2 = 2 * R

P = 128            # partition tile
OTILE = 512        # matmul moving free dim (one PSUM bank of f32)
KT = DIN // P      # 8 k-tiles
OT = DOUT // OTILE # 8 o-tiles
ST = S // P        # 16 s-tiles
SLAB = 512         # xT column slab (4 s-tiles per slab tile)
NSLAB = S // SLAB  # 4
NQ = 4             # o-quarters
NJ = OT // NQ      # 2 o-tiles per quarter
QW = NJ * OTILE    # 1024 columns per quarter


def build_nc():
    nc = bacc.Bacc("TRN2", target_bir_lowering=False, debug=False,
                   num_devices=N_CORES)
    xT = nc.dram_tensor("xT", [DIN, S], F16, kind="ExternalInput").ap()
    WT = nc.dram_tensor("WT", [DIN, DOUT], F16, kind="ExternalInput").ap()
    b_rep = nc.dram_tensor("b_rep", [P, DOUT], F16, kind="ExternalInput").ap()
    A_cat = nc.dram_tensor("A_cat", [P, DIN], F16, kind="ExternalInput").ap()
    B_catT = nc.dram_tensor("B_catT", [P, DOUT], F16, kind="ExternalInput").ap()
    out = nc.dram_tensor("out", [S, DOUT], F16, kind="ExternalOutput").ap()

    with tile.TileContext(nc) as tc:
        with ExitStack() as ctx:
            const = ctx.enter_context(tc.tile_pool(name="const", bufs=1))
            wet_pool = ctx.enter_context(tc.tile_pool(name="wet", bufs=1))
            xt_pool = ctx.enter_context(tc.tile_pool(name="xt", bufs=1))
            out_pool = ctx.enter_context(tc.tile_pool(name="outp", bufs=16))
            psm = ctx.enter_context(tc.tile_pool(name="psm", bufs=6, space="PSUM"))
            psl = ctx.enter_context(tc.tile_pool(name="psl", bufs=2, space="PSUM"))

            # ---- small constants (critical sync queue) ----
            acat = const.tile([P, DIN], F16)
            nc.sync.dma_start(acat[:], A_cat)
            bcatt = const.tile([P, DOUT], F16)
            nc.sync.dma_start(bcatt[:], B_catT)

            # ---- bulk inputs: one SBUF tile per DMA chunk so dependency
            # tracking stays exact (no false whole-tile WAR hazards) ----
            wetq = [[wet_pool.tile([P, QW], F16, tag=f"wetq{k}_{q}",
                                   name=f"wetq{k}_{q}") for q in range(NQ)]
                    for k in range(KT)]
            xts = [[xt_pool.tile([P, SLAB], F16, tag=f"xts{k}_{s}",
                                 name=f"xts{k}_{s}") for s in range(NSLAB)]
                   for k in range(KT)]

            def dma_wet_quarter(q, eng):
                for kt in range(KT):
                    eng.dma_start(wetq[kt][q][:],
                                  WT[ds(kt * P, P), ds(q * QW, QW)])

            def dma_x_slab(sl, eng):
                for kt in range(KT):
                    eng.dma_start(xts[kt][sl][:],
                                  xT[ds(kt * P, P), ts(sl, SLAB)])

            # single sync HWDGE queue, critical prefix first (FIFO gives
            # the prefix full bandwidth; issuing bulk DMAs from ScalarE
            # would block ScalarE's FIFO ahead of the prep evictions)
            dma_wet_quarter(0, nc.sync)
            dma_x_slab(0, nc.sync)
            bias16 = const.tile([P, DOUT], F16)
            nc.sync.dma_start(bias16[:], b_rep)
            dma_x_slab(1, nc.sync)
            dma_wet_quarter(1, nc.sync)
            dma_x_slab(2, nc.sync)
            dma_x_slab(3, nc.sync)
            dma_wet_quarter(2, nc.sync)
            dma_wet_quarter(3, nc.sync)

            # ---- HAM warmup: full-K dummy matmuls un-throttle the PE
            # clock gate while the first real chunks land ----
            warm = const.tile([P, OTILE], F16)
            nc.vector.memset(warm[:], 1.0)
            ident_h = const.tile([P, P], F16)
            make_identity(nc, ident_h)
            psw = psm.tile([P, OTILE], F32, tag="psm")
            NWARM = 24
            for i in range(NWARM):
                nc.tensor.matmul(psw[:], warm[:, ds(0, P)], warm[:],
                                 start=(i == 0), stop=(i == NWARM - 1))
            warm_sink = const.tile([P, 1], F32)
            nc.vector.tensor_copy(warm_sink[:], psw[:, ds(0, 1)])

            def lora_group_fast(q, kt):
                # W_effT chunk fully on PE+ScalarE: psum = LoRA + W.T via
                # an identity matmul, ScalarE evicts f32->f16 in place.
                # Used for the startup-critical quarter (no DVE pacing).
                for j in range(NJ):
                    ot = q * NJ + j
                    wchunk = wetq[kt][q][:, ts(j, OTILE)]
                    pl = psl.tile([P, OTILE], F32, tag="psl",
                                  name=f"plf{q}_{kt}_{j}")
                    nc.tensor.matmul(pl[:], acat[:, ts(kt, P)],
                                     bcatt[:, ts(ot, OTILE)],
                                     start=True, stop=False)
                    nc.tensor.matmul(pl[:], ident_h[:], wchunk,
                                     start=False, stop=True)
                    nc.scalar.copy(wchunk, pl[:])

            def lora_group(q, kt):
                # DVE-add variant, hidden under main compute
                for j in range(NJ):
                    ot = q * NJ + j
                    pl = psl.tile([P, OTILE], F32, tag="psl",
                                  name=f"pl{q}_{kt}_{j}")
                    nc.tensor.matmul(pl[:], acat[:, ts(kt, P)],
                                     bcatt[:, ts(ot, OTILE)],
                                     start=True, stop=True)
                    wchunk = wetq[kt][q][:, ts(j, OTILE)]
                    nc.vector.tensor_tensor(wchunk, pl[:], wchunk,
                                            mybir.AluOpType.add)

            def main_quarter(q, interleave_lora_q=None):
                # out[s, o-quarter] = x @ W_effT + bias; slot the next
                # quarter's LoRA prep between s-tiles to keep PE dense
                for st in range(ST):
                    if interleave_lora_q is not None and 4 <= st < 4 + KT:
                        lora_group(interleave_lora_q, st - 4)
                    pos = [psm.tile([P, OTILE], F32, tag="psm",
                                    name=f"pos{q}_{st}_{j}")
                           for j in range(NJ)]
                    for kt in range(KT):
                        xchunk = xts[kt][st // 4][:, ds((st % 4) * P, P)]
                        for j in range(NJ):
                            nc.tensor.matmul(pos[j][:], xchunk,
                                             wetq[kt][q][:, ts(j, OTILE)],
                                             start=(kt == 0),
                                             stop=(kt == KT - 1))
                    for j in range(NJ):
                        ot = q * NJ + j
                        osb = out_pool.tile([P, OTILE], F16)
                        nc.vector.tensor_tensor(osb[:], pos[j][:],
                                                bias16[:, ts(ot, OTILE)],
                                                mybir.AluOpType.add)
                        nc.sync.dma_start(out[ts(st, P), ts(ot, OTILE)],
                                          osb[:])

            for kt in range(KT):
                lora_group_fast(0, kt)
            main_quarter(0, interleave_lora_q=1)
            main_quarter(1, interleave_lora_q=2)
            main_quarter(2, interleave_lora_q=3)
            main_quarter(3)

    nc.compile()
    return nc


_NC_CACHE = None


def _get_nc():
    global _NC_CACHE
    if _NC_CACHE is None:
        _NC_CACHE = build_nc()
    return _NC_CACHE


def make_in_maps(x, W, b, global_A, global_B, local_A, local_B):
    x = np.asarray(x, dtype=np.float32)
    W = np.asarray(W, dtype=np.float32)
    b = np.asarray(b, dtype=np.float32)
    xT = np.ascontiguousarray(x.transpose(0, 2, 1).astype(np.float16))
    WT = np.ascontiguousarray(W.T.astype(np.float16))
    # rank-16 LoRA factors, zero-padded to K=128 so the device matmuls
    # run full-array (keeps the PE HAM clock gate warm; padded rows are
    # zero in both operands so the result is exact)
    A_cat = np.zeros((P, DIN), dtype=np.float16)
    A_cat[:R2] = (SCALE * np.concatenate(
        [np.asarray(global_A), np.asarray(local_A)], axis=0)).astype(np.float16)
    B_catT = np.zeros((P, DOUT), dtype=np.float16)
    B_catT[:R2] = np.concatenate(
        [np.asarray(global_B).T, np.asarray(local_B).T], axis=0).astype(np.float16)
    b_rep = np.ascontiguousarray(
        np.broadcast_to(b.astype(np.float16), (P, DOUT)))
    return [
        {"xT": xT[i], "WT": WT, "b_rep": b_rep, "A_cat": A_cat,
         "B_catT": B_catT}
        for i in range(N_CORES)
    ]


def kernel(x, W, b, global_A, global_B, local_A, local_B):
    nc = _get_nc()
    in_maps = make_in_maps(x, W, b, global_A, global_B, local_A, local_B)
    res = run_bass_kernel_spmd(nc, in_maps, list(range(N_CORES))).results
    return np.stack([res[i]["out"].astype(np.float32) for i in range(N_CORES)],
                    axis=0)


# revision 21
# speedup vs baseline: 1.1654x; 1.0030x over previous
"""LoRALinear kernel for Trainium2 (8 NeuronCores, SPMD data-parallel).

Computes out = x @ W.T + b + SCALE*((x@gA.T)@gB.T + (x@lA.T)@lB.T)
  x: [8, 2048, 1024] f32, W: [4096, 1024], b: [4096]
  gA/lA: [8, 1024], gB/lB: [4096, 8]  ->  out: [8, 2048, 4096] f32

Strategy (per core, one batch of x each). Host ships layout-marshaled
fp16 operands (x.T per core, W.T replicated, zero-padded rank-16 LoRA
factors, bias row replicated to 128 partitions); every FLOP runs on
device:
  1. Warmup: a few full-K dummy matmuls un-throttle the PE HAM clock
     gate (1.2 -> 2.4 GHz) while the first W.T/x.T chunks DMA in
     (split across both HWDGE queues: critical prefix on sync,
     bulk on the scalar engine's queue).
  2. W_effT = W.T + (SCALE*A_cat).T @ B_catT built in place over the
     W.T tiles. The o-quarter needed first is merged entirely on
     PE+ScalarE (LoRA matmul, then identity matmul accumulating W.T
     into the same psum, ScalarE f32->f16 eviction) so the slow DVE
     psum-add never gates the start; later quarters use the DVE add
     path, hidden under main compute.
  3. Main matmul in four o-quarter passes: per 128-row s-tile,
     accumulate psum over 8 k-chunks with the xT chunk stationary
     across 2 o-tiles (LDWEIGHTS amortized). Every W/x chunk is its
     own SBUF tile so dependency tracking stays exact. LoRA prep for
     quarter q+1 is slotted between s-tiles of quarter q. PSUM pools
     are split (prep 2 banks / main 6) to avoid false coupling. DVE
     evicts with fused bias add to fp16; host upcasts to f32.

fp16 operands and fp16 output staging give ~6e-4 absmax relative
error vs the f32 reference; accumulation stays f32 in PSUM.
"""
import numpy as np
from contextlib import ExitStack

import concourse.bass as bass
import concourse.tile as tile
from concourse import bacc, mybir
from concourse.bass import ts, ds
from concourse.bass_utils import run_bass_kernel_spmd
from concourse.masks import make_identity

F32 = mybir.dt.float32
F16 = mybir.dt.float16

N_CORES = 8
B, S, DIN, DOUT, R = 8, 2048, 1024, 4096, 8
SCALE = 16.0 / 8
R2 = 2 * R

P = 128            # partition tile
OTILE = 512        # matmul moving free dim (one PSUM bank of f32)
KT = DIN // P      # 8 k-tiles
OT = DOUT // OTILE # 8 o-tiles
ST = S // P        # 16 s-tiles
SLAB = 512         # xT column slab (4 s-tiles per slab tile)
NSLAB = S // SLAB  # 4
NQ = 4             # o-quarters
NJ = OT // NQ      # 2 o-tiles per quarter
QW = NJ * OTILE    # 1024 columns per quarter


def build_nc():
    nc = bacc.Bacc("TRN2", target_bir_lowering=False, debug=False,
                   num_devices=N_CORES)
    xT = nc.dram_tensor("xT", [DIN, S], F16, kind="ExternalInput").ap()
    WT = nc.dram_tensor("WT", [DIN, DOUT], F16, kind="ExternalInput").ap()
    b_rep = nc.dram_tensor("b_rep", [P, DOUT], F16, kind="ExternalInput").ap()
    A_cat = nc.dram_tensor("A_cat", [P, DIN], F16, kind="ExternalInput").ap()
    B_catT = nc.dram_tensor("B_catT", [P, DOUT], F16, kind="ExternalInput").ap()
    out = nc.dram_tensor("out", [S, DOUT], F16, kind="ExternalOutput").ap()

    with tile.TileContext(nc) as tc:
        with ExitStack() as ctx:
            const = ctx.enter_context(tc.tile_pool(name="const", bufs=1))
            wet_pool = ctx.enter_context(tc.tile_pool(name="wet", bufs=1))
            xt_pool = ctx.enter_context(tc.tile_pool(name="xt", bufs=1))
            out_pool = ctx.enter_context(tc.tile_pool(name="outp", bufs=16))
            psm = ctx.enter_context(tc.tile_pool(name="psm", bufs=6, space="PSUM"))
            psl = ctx.enter_context(tc.tile_pool(name="psl", bufs=2, space="PSUM"))

            # ---- small constants (critical sync queue) ----
            acat = const.tile([P, DIN], F16)
            nc.sync.dma_start(acat[:], A_cat)
            bcatt = const.tile([P, DOUT], F16)
            nc.sync.dma_start(bcatt[:], B_catT)

            # ---- bulk inputs: one SBUF tile per DMA chunk so dependency
            # tracking stays exact (no false whole-tile WAR hazards) ----
            wetq = [[wet_pool.tile([P, QW], F16, tag=f"wetq{k}_{q}",
                                   name=f"wetq{k}_{q}") for q in range(NQ)]
                    for k in range(KT)]
            xts = [[xt_pool.tile([P, SLAB], F16, tag=f"xts{k}_{s}",
                                 name=f"xts{k}_{s}") for s in range(NSLAB)]
                   for k in range(KT)]

            def dma_wet_quarter(q, eng):
                for kt in range(KT):
                    eng.dma_start(wetq[kt][q][:],
                                  WT[ds(kt * P, P), ds(q * QW, QW)])

            def dma_x_slab(sl, eng):
                for kt in range(KT):
                    eng.dma_start(xts[kt][sl][:],
                                  xT[ds(kt * P, P), ts(sl, SLAB)])

            # single sync HWDGE queue, critical prefix first (FIFO gives
            # the prefix full bandwidth; issuing bulk DMAs from ScalarE
            # would block ScalarE's FIFO ahead of the prep evictions)
            dma_wet_quarter(0, nc.sync)
            dma_x_slab(0, nc.sync)
            bias16 = const.tile([P, DOUT], F16)
            nc.sync.dma_start(bias16[:], b_rep)
            dma_x_slab(1, nc.sync)
            dma_wet_quarter(1, nc.sync)
            dma_x_slab(2, nc.sync)
            dma_x_slab(3, nc.sync)
            dma_wet_quarter(2, nc.sync)
            dma_wet_quarter(3, nc.sync)

            # ---- HAM warmup: full-K dummy matmuls un-throttle the PE
            # clock gate while the first real chunks land ----
            warm = const.tile([P, OTILE], F16)
            nc.vector.memset(warm[:], 1.0)
            ident_h = const.tile([P, P], F16)
            make_identity(nc, ident_h)
            psw = psm.tile([P, OTILE], F32, tag="psm")
            NWARM = 24
            for i in range(NWARM):
                nc.tensor.matmul(psw[:], warm[:, ds(0, P)], warm[:],
                                 start=(i == 0), stop=(i == NWARM - 1))
            warm_sink = const.tile([P, 1], F32)
            nc.vector.tensor_copy(warm_sink[:], psw[:, ds(0, 1)])

            def lora_group_fast(q, kt):
                # W_effT chunk fully on PE+ScalarE: psum = LoRA + W.T via
                # an identity matmul, ScalarE evicts f32->f16 in place.
                # Used for the startup-critical quarter (no DVE pacing).
                for j in range(NJ):
                    ot = q * NJ + j
                    wchunk = wetq[kt][q][:, ts(j, OTILE)]
                    pl = psl.tile([P, OTILE], F32, tag="psl",
                                  name=f"plf{q}_{kt}_{j}")
                    nc.tensor.matmul(pl[:], acat[:, ts(kt, P)],
                                     bcatt[:, ts(ot, OTILE)],
                                     start=True, stop=False)
                    nc.tensor.matmul(pl[:], ident_h[:], wchunk,
                                     start=False, stop=True)
                    nc.scalar.copy(wchunk, pl[:])

            def lora_group(q, kt):
                # DVE-add variant, hidden under main compute
                for j in range(NJ):
                    ot = q * NJ + j
                    pl = psl.tile([P, OTILE], F32, tag="psl",
                                  name=f"pl{q}_{kt}_{j}")
                    nc.tensor.matmul(pl[:], acat[:, ts(kt, P)],
                                     bcatt[:, ts(ot, OTILE)],
                                     start=True, stop=True)
                    wchunk = wetq[kt][q][:, ts(j, OTILE)]
                    nc.vector.tensor_tensor(wchunk, pl[:], wchunk,
                                            mybir.AluOpType.add)

            def main_quarter(q, interleave_lora_q=None):
                # out[s, o-quarter] = x @ W_effT + bias; slot the next
                # quarter's LoRA prep between s-tiles to keep PE dense
                for st in range(ST):
                    if interleave_lora_q is not None and 4 <= st < 4 + KT:
                        lora_group(interleave_lora_q, st - 4)
                    pos = [psm.tile([P, OTILE], F32, tag="psm",
                                    name=f"pos{q}_{st}_{j}")
                           for j in range(NJ)]
                    for kt in range(KT):
                        xchunk = xts[kt][st // 4][:, ds((st % 4) * P, P)]
                        for j in range(NJ):
                            nc.tensor.matmul(pos[j][:], xchunk,
                                             wetq[kt][q][:, ts(j, OTILE)],
                                             start=(kt == 0),
                                             stop=(kt == KT - 1))
                    for j in range(NJ):
                        ot = q * NJ + j
                        osb = out_pool.tile([P, OTILE], F16)
                        nc.vector.tensor_tensor(osb[:], pos[j][:],
                                                bias16[:, ts(ot, OTILE)],
                                                mybir.AluOpType.add)
                        nc.sync.dma_start(out[ts(st, P), ts(ot, OTILE)],
                                          osb[:])

            for kt in range(KT):
                lora_group_fast(0, kt)
            main_quarter(0, interleave_lora_q=1)
            main_quarter(1, interleave_lora_q=2)
            main_quarter(2, interleave_lora_q=3)
            main_quarter(3)

    nc.compile()
    return nc


_NC_CACHE = None


def _get_nc():
    global _NC_CACHE
    if _NC_CACHE is None:
        _NC_CACHE = build_nc()
    return _NC_CACHE


def make_in_maps(x, W, b, global_A, global_B, local_A, local_B):
    x = np.asarray(x, dtype=np.float32)
    W = np.asarray(W, dtype=np.float32)
    b = np.asarray(b, dtype=np.float32)
    xT = np.ascontiguousarray(x.transpose(0, 2, 1).astype(np.float16))
    WT = np.ascontiguousarray(W.T.astype(np.float16))
    # rank-16 LoRA factors, zero-padded to K=128 so the device matmuls
    # run full-array (keeps the PE HAM clock gate warm; padded rows are
    # zero in both operands so the result is exact)
    A_cat = np.zeros((P, DIN), dtype=np.float16)
    A_cat[:R2] = (SCALE * np.concatenate(
        [np.asarray(global_A), np.asarray(local_A)], axis=0)).astype(np.float16)
    B_catT = np.zeros((P, DOUT), dtype=np.float16)
    B_catT[:R2] = np.concatenate(
        [np.asarray(global_B).T, np.asarray(local_B).T], axis=0).astype(np.float16)
    b_rep = np.ascontiguousarray(
        np.broadcast_to(b.astype(np.float16), (P, DOUT)))
    return [
        {"xT": xT[i], "WT": WT, "b_rep": b_rep, "A_cat": A_cat,
         "B_catT": B_catT}
        for i in range(N_CORES)
    ]


def kernel(x, W, b, global_A, global_B, local_A, local_B):
    nc = _get_nc()
    in_maps = make_in_maps(x, W, b, global_A, global_B, local_A, local_B)
    res = run_bass_kernel_spmd(nc, in_maps, list(range(N_CORES))).results
    return np.stack([res[i]["out"].astype(np.float32) for i in range(N_CORES)],
                    axis=0)
